# revision 1
# baseline (speedup 1.0000x reference)
"""Trainium2 Bass kernel for an MoE transformer block (attention + top-2 MoE FFN).

Sharding across 8 NeuronCores:
  - sequence-parallel attention: core r owns tokens [256r, 256r+256)
  - expert-parallel MoE: core r owns experts {2r, 2r+1}
  - AllGather K^T / V-hat / moe_in / combine-weights, ReduceScatter expert outputs.
"""

import sys

for p in ("/opt/trn_rl_repo",):
    if p not in sys.path:
        sys.path.insert(0, p)

import numpy as np

from concourse import bass, mybir
import concourse.tile as tile
from concourse.masks import make_identity
from concourse.bass_utils import run_bass_kernel_spmd

# --- workaround: this walrus build caps sync-waits per CTRL instruction at 2.
# Tile's kernel-tail drain can carry 3+; split the waits across extra drains.
import concourse.tile as _tile_mod


def _split_drain_and_barrier(self, tick_clock, wait_clock):
    nc = self.nc
    drain_inst = nc.sync.drain()
    wait_clock.add_sem_waits(
        drain_inst.ins, _tile_mod.ScopedClock({None: tick_clock.global_clock})
    )
    si = drain_inst.ins.sync_info
    if si is not None and si.on_wait and len(si.on_wait) > 1:
        waits = list(si.on_wait)
        si.on_wait = waits[:1]
        rest = waits[1:]
        while rest:
            d2 = nc.sync.drain()
            d2.ins.sync_info = mybir.SyncInfo(on_update=[], on_wait=rest[:1])
            rest = rest[1:]
    nc.all_engine_barrier()
    assert self.sems is not None
    popped = nc._tile_sem_poison_stack.pop()
    assert popped is self._sem_poison
    nc.clear_and_free_semaphores(list(self.sems.allocated().values()))
    nc.all_engine_barrier()


_tile_mod.TileContext._drain_and_barrier = _split_drain_and_barrier

# --- workaround #2: the same walrus build allows only ONE sync-wait per
# instruction. Tile's stage-1B freely emits several. Rewrite the serialized
# BIR before compilation: move excess waits onto same-engine NoOp carriers
# inserted immediately before the instruction (identical AND semantics,
# since semaphores are monotonic).
import json as _json
import concourse.bass_utils as _bu
import concourse.bass2jax as _b2j

_WAIT_LIMIT = 1


def _split_sync_waits_json(bir_bytes):
    bir = _json.loads(bir_bytes)
    cnt = 0
    for f in bir["functions"]:
        for b in f["blocks"]:
            out = []
            for ins in b["instructions"]:
                si = ins.get("sync_info")
                waits = (si or {}).get("on_wait") or []
                if len(waits) > _WAIT_LIMIT and ins.get("engine") not in (
                    None, "Unassigned"):
                    keep = waits[-_WAIT_LIMIT:]
                    extra = waits[:-_WAIT_LIMIT]
                    while extra:
                        chunk, extra = extra[:_WAIT_LIMIT], extra[_WAIT_LIMIT:]
                        cnt += 1
                        out.append({
                            "debug": ins.get("debug", 0),
                            "engine": ins["engine"],
                            "ins": [],
                            "outs": [],
                            "name": f"{ins['name']}-w{cnt}",
                            "opcode": "NoOp",
                            "sync_info": {"on_update": [], "on_wait": chunk},
                        })
                    si["on_wait"] = keep
                out.append(ins)
            b["instructions"] = out
    return _json.dumps(bir).encode()


_orig_compile_bir_kernel = _bu.compile_bir_kernel


def _patched_compile_bir_kernel(bir_json, tmpdir, neff_name="file.neff"):
    return _orig_compile_bir_kernel(
        _split_sync_waits_json(bir_json), tmpdir, neff_name=neff_name)


_bu.compile_bir_kernel = _patched_compile_bir_kernel
_b2j.compile_bir_kernel = _patched_compile_bir_kernel

F32 = mybir.dt.float32
BF16 = mybir.dt.bfloat16
I32 = mybir.dt.int32

P = 128
T = 2048          # total tokens
HID = 768
NQ = 12
NKV = 3
HD = 64
E = 16
FF = 1536
EPS = 1e-6
NCORES = 8
TOK = T // NCORES        # 256 tokens per core
KC = T // P              # 16 key chunks of 128
EPL = E // NCORES        # 2 experts per core
CAP = 384                # per-expert token capacity (max observed load ~296)
CT = CAP // P            # capacity tiles (3)
CF = CAP // 16           # sparse-gather output free size (24)
SENT = T                 # sentinel row index (2048) in the padded moe buffer
QKVD = (NQ + 2 * NKV) * HD  # 1152
VHAT = NKV * (HD + 1)       # 195
RG = [list(range(NCORES))]


def _build_program():
    nc = bass.Bass()

    x_in = nc.declare_dram_parameter("x_chunk", [TOK, HID], F32, isOutput=False)
    wqkv_in = nc.declare_dram_parameter("w_qkv", [HID, QKVD], F32, isOutput=False)
    wout_in = nc.declare_dram_parameter("w_out", [NQ * HD, HID], F32, isOutput=False)
    wrout_in = nc.declare_dram_parameter("w_router", [HID, E], F32, isOutput=False)
    wgu_in = nc.declare_dram_parameter("w_gu", [EPL, HID, 2 * FF], BF16, isOutput=False)
    wdn_in = nc.declare_dram_parameter("w_dn", [EPL, FF, HID], BF16, isOutput=False)
    nw1_in = nc.declare_dram_parameter("nw1", [P, HID], F32, isOutput=False)
    nw2_in = nc.declare_dram_parameter("nw2", [P, HID], F32, isOutput=False)
    cos_in = nc.declare_dram_parameter("rope_cos", [TOK, HD // 2], F32, isOutput=False)
    sin_in = nc.declare_dram_parameter("rope_sin", [TOK, HD // 2], F32, isOutput=False)
    # causal mask, transposed orientation: mask[kc, l, q] for this core's 256 queries
    mask_in = nc.declare_dram_parameter("maskT", [KC, P, TOK], BF16, isOutput=False)
    # one-hot selectors for this core's two expert columns of combine [128,16] each
    sel_in = nc.declare_dram_parameter("sel", [EPL, P, E], F32, isOutput=False)
    out_ext = nc.declare_dram_parameter("out_chunk", [TOK, HID], F32, isOutput=True)

    with tile.TileContext(nc) as tc:
        with (
            tc.tile_pool(name="const", bufs=1) as constp,
            tc.tile_pool(name="dram", bufs=1, space="DRAM") as dramp,
            tc.tile_pool(name="lp", bufs=1) as lp,
            tc.tile_pool(name="sb2", bufs=2) as sb2,
            tc.tile_pool(name="ps", bufs=4, space="PSUM") as ps,
            tc.tile_pool(name="ps_acc", bufs=2, space="PSUM") as ps_acc,
        ):
            ident = constp.tile([P, P], F32, name="ident", tag="ident")
            make_identity(nc, ident[:])
            ident_bf = constp.tile([P, P], BF16, name="ident_bf", tag="ident_bf")
            nc.vector.tensor_copy(ident_bf[:], ident[:])
            ones_row = constp.tile([1, P], F32, name="ones_row", tag="ones_row")
            nc.vector.memset(ones_row[:], 1.0)
            eps_t = constp.tile([P, 1], F32, name="eps_t", tag="eps_t")
            nc.vector.memset(eps_t[:], EPS)

            # ---- internal DRAM (collective + scratch) ----
            agk_in = dramp.tile([NKV * HD, TOK], BF16, name="agk_in", tag="agk_in")
            agk_out = dramp.tile([NCORES * NKV * HD, TOK], BF16, name="agk_out",
                                 tag="agk_out", addr_space="Shared")
            agv_in = dramp.tile([TOK, VHAT], BF16, name="agv_in", tag="agv_in")
            agv_out = dramp.tile([T, VHAT], BF16, name="agv_out", tag="agv_out",
                                 addr_space="Shared")
            agm_in = dramp.tile([TOK, HID], BF16, name="agm_in", tag="agm_in")
            # not Shared: the dummy sentinel row 2048 needs a second writer
            agm_out = dramp.tile([T + 1, HID], BF16, name="agm_out", tag="agm_out")
            agc_in = dramp.tile([TOK, E], F32, name="agc_in", tag="agc_in")
            agc_out = dramp.tile([T, E], F32, name="agc_out", tag="agc_out",
                                 addr_space="Shared")
            partial = dramp.tile([T + 1, HID], BF16, name="partial", tag="partial")
            rs_out = dramp.tile([TOK, HID], BF16, name="rs_out", tag="rs_out")
            colbuf = dramp.tile([T], F32, name="colbuf", tag="colbuf")
            scr_idx = dramp.tile([EPL, CAP], F32, name="scr_idx", tag="scr_idx")
            scr_w = dramp.tile([EPL, CAP], F32, name="scr_w", tag="scr_w")

            # residual stream tiles live across both phases
            h_sb = [lp.tile([P, HID], F32, name=f"h{t}", tag=f"h{t}") for t in range(2)]
            comb_sb = [lp.tile([P, E], F32, name=f"comb{t}", tag=f"comb{t}")
                       for t in range(2)]

            # zero the scatter target (and dummy row of the moe buffer)
            zrow = constp.tile([P, HID], BF16, name="zrow", tag="zrow")
            nc.vector.memset(zrow[:], 0.0)
            for i in range(T // P):
                nc.sync.dma_start(partial[i * P:(i + 1) * P, :], zrow[:])
            nc.sync.dma_start(partial[T:T + 1, :], zrow[0:1, :])
            nc.sync.dma_start(agm_out[T:T + 1, :], zrow[0:1, :])

            def transpose_128(dst_ap, src_ap):
                """dst[f, t] = src[t, f] for one [128, <=128] block via PE."""
                is_bf = src_ap.dtype == BF16
                pt = ps.tile([P, P], BF16 if is_bf else F32, name="pt", tag="ps")
                fsz = src_ap.shape[1]
                idn = ident_bf if is_bf else ident
                nc.tensor.matmul(out=pt[:fsz, :P], lhsT=src_ap, rhs=idn[:, :P],
                                 start=True, stop=True, is_transpose=True)
                nc.vector.tensor_copy(dst_ap, pt[:fsz, :P])

            def rms_norm_tiles(src_tiles, w_tile, dst_tiles, tagp):
                for t, (src, dst) in enumerate(zip(src_tiles, dst_tiles)):
                    sq = sb2.tile([P, HID], F32, name="rms_sq", tag="rms_sq")
                    ssum = sb2.tile([P, 1], F32, name="rms_ss", tag="rms_ss")
                    nc.scalar.activation(sq[:], src[:],
                                         mybir.ActivationFunctionType.Square,
                                         accum_out=ssum[:])
                    sroot = sb2.tile([P, 1], F32, name="rms_sr", tag="rms_sr")
                    nc.scalar.activation(sroot[:], ssum[:],
                                         mybir.ActivationFunctionType.Sqrt,
                                         bias=eps_t[:], scale=1.0 / HID)
                    rs = sb2.tile([P, 1], F32, name="rms_rs", tag="rms_rs")
                    nc.vector.reciprocal(rs[:], sroot[:])
                    nc.vector.tensor_mul(dst[:], src[:], rs[:].to_broadcast([P, HID]))
                    nc.vector.tensor_mul(dst[:], dst[:], w_tile[:])

            # ======================= attention phase =======================
            with tc.tile_pool(name="attp", bufs=1) as attp, \
                 tc.tile_pool(name="att3", bufs=3) as att3:
                nw1_sb = attp.tile([P, HID], F32, name="nw1", tag="nw1")
                nc.sync.dma_start(nw1_sb[:], nw1_in[:])
                nw2_sb = attp.tile([P, HID], F32, name="nw2", tag="nw2")
                nc.sync.dma_start(nw2_sb[:], nw2_in[:])
                wrout_sb = [attp.tile([P, E], F32, name=f"wrout{k}", tag=f"wrout{k}")
                            for k in range(HID // P)]
                for k in range(HID // P):
                    nc.sync.dma_start(wrout_sb[k][:], wrout_in[k * P:(k + 1) * P, :])
                cos_sb = [attp.tile([P, HD // 2], F32, name=f"cos{t}", tag=f"cos{t}")
                          for t in range(2)]
                sin_sb = [attp.tile([P, HD // 2], F32, name=f"sin{t}", tag=f"sin{t}")
                          for t in range(2)]
                for t in range(2):
                    nc.sync.dma_start(cos_sb[t][:], cos_in[t * P:(t + 1) * P, :])
                    nc.sync.dma_start(sin_sb[t][:], sin_in[t * P:(t + 1) * P, :])
                mask_sb = attp.tile([P, KC * TOK], BF16, name="mask", tag="mask")
                for kc in range(KC):
                    nc.sync.dma_start(mask_sb[:, kc * TOK:(kc + 1) * TOK], mask_in[kc])

                x_sb = [attp.tile([P, HID], F32, name=f"x{t}", tag=f"x{t}")
                        for t in range(2)]
                for t in range(2):
                    nc.sync.dma_start(x_sb[t][:], x_in[t * P:(t + 1) * P, :])

                # rms_norm 1, x_norm^T, qkv projection — in a sub-scope so the
                # space is reclaimed for w_out / router tensors afterwards
                subA = tc.tile_pool(name="subA", bufs=1)
                subA_pool = subA.__enter__()
                wqkv_sb = [subA_pool.tile([P, QKVD], F32, name=f"wqkv{k}",
                                          tag=f"wqkv{k}") for k in range(HID // P)]
                for k in range(HID // P):
                    nc.sync.dma_start(wqkv_sb[k][:], wqkv_in[k * P:(k + 1) * P, :])
                xn_sb = [subA_pool.tile([P, HID], F32, name=f"xn{t}", tag=f"xn{t}")
                         for t in range(2)]
                rms_norm_tiles(x_sb, nw1_sb, xn_sb, "rms1")
                xnT = subA_pool.tile([P, (HID // P) * TOK], F32, name="xnT", tag="xnT")
                for t in range(2):
                    for k in range(HID // P):
                        transpose_128(xnT[:, k * TOK + t * P:k * TOK + (t + 1) * P],
                                      xn_sb[t][:, k * P:(k + 1) * P])

                # qkv = xn @ w_qkv  (token-major [256, 1152])
                qkv_sb = [subA_pool.tile([P, QKVD], F32, name=f"qkv{t}", tag=f"qkv{t}")
                          for t in range(2)]
                for t in range(2):
                    for n in range(3):
                        pq = ps.tile([P, 384], F32, name="pq", tag="ps")
                        for k in range(HID // P):
                            nc.tensor.matmul(
                                out=pq[:],
                                lhsT=xnT[:, k * TOK + t * P:k * TOK + (t + 1) * P],
                                rhs=wqkv_sb[k][:, n * 384:(n + 1) * 384],
                                start=(k == 0), stop=(k == HID // P - 1))
                        nc.vector.tensor_copy(qkv_sb[t][:, n * 384:(n + 1) * 384], pq[:])

                # RoPE on q and k (interleaved pairs)
                qr_sb = [attp.tile([P, NQ * HD], F32, name=f"qr{t}", tag=f"qr{t}")
                         for t in range(2)]
                kr_sb = [attp.tile([P, NKV * HD], F32, name=f"kr{t}", tag=f"kr{t}")
                         for t in range(2)]
                for t in range(2):
                    for (src_off, nh, dst) in ((0, NQ, qr_sb[t]),
                                               (NQ * HD, NKV, kr_sb[t])):
                        src4 = qkv_sb[t][:, src_off:src_off + nh * HD].rearrange(
                            "p (h i two) -> p h i two", two=2, i=HD // 2)
                        dst4 = dst[:].rearrange("p (h i two) -> p h i two",
                                                two=2, i=HD // 2)
                        ev, od = src4[:, :, :, 0], src4[:, :, :, 1]
                        cosb = cos_sb[t][:].rearrange("p i -> p () i").to_broadcast(
                            [P, nh, HD // 2])
                        sinb = sin_sb[t][:].rearrange("p i -> p () i").to_broadcast(
                            [P, nh, HD // 2])
                        ta = sb2.tile([P, nh * HD // 2], F32, name="ra", tag="ra")
                        tb = sb2.tile([P, nh * HD // 2], F32, name="rb", tag="rb")
                        ta3 = ta[:].rearrange("p (h i) -> p h i", i=HD // 2)
                        tb3 = tb[:].rearrange("p (h i) -> p h i", i=HD // 2)
                        nc.vector.tensor_mul(ta3, ev, cosb)
                        nc.vector.tensor_mul(tb3, od, sinb)
                        nc.vector.tensor_sub(dst4[:, :, :, 0], ta3, tb3)
                        nc.vector.tensor_mul(ta3, ev, sinb)
                        nc.vector.tensor_mul(tb3, od, cosb)
                        nc.vector.tensor_add(dst4[:, :, :, 1], ta3, tb3)

                # local K^T -> AllGather
                kTl = [attp.tile([HD, TOK], BF16, name=f"kTl{g}", tag=f"kTl{g}")
                       for g in range(NKV)]
                for t in range(2):
                    for g in range(NKV):
                        transpose_128(kTl[g][:, t * P:(t + 1) * P],
                                      kr_sb[t][:, g * HD:(g + 1) * HD])
                for g in range(NKV):
                    nc.sync.dma_start(agk_in[g * HD:(g + 1) * HD, :], kTl[g][:])
                nc.gpsimd.collective_compute(
                    "AllGather", mybir.AluOpType.bypass,
                    ins=[agk_in[:]], outs=[agk_out[:]], replica_groups=RG)

                # local V-hat (v columns + ones col per head) -> AllGather
                vh_sb = [attp.tile([P, VHAT], BF16, name=f"vh{t}", tag=f"vh{t}")
                         for t in range(2)]
                for t in range(2):
                    for g in range(NKV):
                        nc.vector.tensor_copy(
                            vh_sb[t][:, g * (HD + 1):g * (HD + 1) + HD],
                            qkv_sb[t][:, (NQ + NKV) * HD + g * HD:
                                      (NQ + NKV) * HD + (g + 1) * HD])
                        nc.vector.memset(
                            vh_sb[t][:, g * (HD + 1) + HD:(g + 1) * (HD + 1)], 1.0)
                    nc.sync.dma_start(agv_in[t * P:(t + 1) * P, :], vh_sb[t][:])
                nc.gpsimd.collective_compute(
                    "AllGather", mybir.AluOpType.bypass,
                    ins=[agv_in[:]], outs=[agv_out[:]], replica_groups=RG)

                subA.__exit__(None, None, None)
                subC = tc.tile_pool(name="subC", bufs=1)
                subC_pool = subC.__enter__()
                wout_sb = [subC_pool.tile([HD, HID], F32, name=f"wout{k}",
                                          tag=f"wout{k}") for k in range(NQ)]
                for k in range(NQ):
                    nc.sync.dma_start(wout_sb[k][:], wout_in[k * HD:(k + 1) * HD, :])

                # q^T per head
                qTh = [attp.tile([HD, TOK], BF16, name=f"qTh{h}", tag=f"qTh{h}")
                       for h in range(NQ)]
                for t in range(2):
                    for h in range(NQ):
                        transpose_128(qTh[h][:, t * P:(t + 1) * P],
                                      qr_sb[t][:, h * HD:(h + 1) * HD])

                # gathered K^T / V-hat into SBUF
                kTg = [attp.tile([HD, T], BF16, name=f"kTg{g}", tag=f"kTg{g}")
                       for g in range(NKV)]
                for g in range(NKV):
                    for j in range(NCORES):
                        nc.sync.dma_start(
                            kTg[g][:, j * TOK:(j + 1) * TOK],
                            agk_out[j * NKV * HD + g * HD:
                                    j * NKV * HD + (g + 1) * HD, :])
                vhg = [[attp.tile([P, HD + 1], BF16, name=f"vhg{kc}_{g}",
                                  tag=f"vhg{kc}_{g}") for g in range(NKV)]
                       for kc in range(KC)]
                for kc in range(KC):
                    for g in range(NKV):
                        nc.sync.dma_start(
                            vhg[kc][g][:],
                            agv_out[kc * P:(kc + 1) * P,
                                    g * (HD + 1):(g + 1) * (HD + 1)])

                # attention: scoresT orientation, exp, mask, V-hat matmul
                aoTh = [attp.tile([HD, TOK], F32, name=f"aoTh{h}", tag=f"aoTh{h}")
                        for h in range(NQ)]
                for h in range(NQ):
                    g = h // (NQ // NKV)
                    po = ps_acc.tile([HD + 1, TOK], F32, name="po", tag="acc")
                    for kcp in range(KC // 2):
                        pscore = ps.tile([P, 2 * TOK], F32, name="psc", tag="ps")
                        for half in range(2):
                            kc = kcp * 2 + half
                            nc.tensor.matmul(
                                out=pscore[:, half * TOK:(half + 1) * TOK],
                                lhsT=kTg[g][:, kc * P:(kc + 1) * P],
                                rhs=qTh[h][:],
                                start=True, stop=True)
                        et = att3.tile([P, 2 * TOK], BF16, name="et", tag="et")
                        nc.scalar.activation(et[:], pscore[:],
                                             mybir.ActivationFunctionType.Exp,
                                             scale=1.0 / np.sqrt(HD))
                        nc.vector.tensor_mul(
                            et[:], et[:],
                            mask_sb[:, kcp * 2 * TOK:(kcp + 1) * 2 * TOK])
                        for half in range(2):
                            kc = kcp * 2 + half
                            nc.tensor.matmul(
                                out=po[:],
                                lhsT=vhg[kc][g][:],
                                rhs=et[:, half * TOK:(half + 1) * TOK],
                                start=(kc == 0), stop=(kc == KC - 1))
                    # normalize rows 0:64 by the ones-column sum (row 64)
                    r64 = sb2.tile([HD + 1, TOK], F32, name="r64", tag="r64")
                    nc.vector.reciprocal(r64[HD:HD + 1, :], po[HD:HD + 1, :])
                    rsum = sb2.tile([1, TOK], F32, name="rsum", tag="rsum")
                    nc.sync.dma_start(rsum[:], r64[HD:HD + 1, :])
                    pb = ps.tile([HD, TOK], F32, name="pb", tag="ps")
                    nc.tensor.matmul(out=pb[:], lhsT=ones_row[:, :HD], rhs=rsum[:],
                                     start=True, stop=True)
                    pbs = sb2.tile([HD, TOK], F32, name="pbs", tag="pbs")
                    nc.scalar.copy(pbs[:], pb[:])
                    nc.vector.tensor_mul(aoTh[h][:], po[:HD, :], pbs[:])

                # out-proj + residual -> h
                for t in range(2):
                    for n in range(2):
                        pho = ps.tile([P, 384], F32, name="pho", tag="ps")
                        for k in range(NQ):
                            nc.tensor.matmul(
                                out=pho[:],
                                lhsT=aoTh[k][:, t * P:(t + 1) * P],
                                rhs=wout_sb[k][:, n * 384:(n + 1) * 384],
                                start=(k == 0), stop=(k == NQ - 1))
                        nc.vector.tensor_add(h_sb[t][:, n * 384:(n + 1) * 384],
                                             pho[:], x_sb[t][:, n * 384:(n + 1) * 384])

                # rms_norm 2 + router
                mi_sb = [subC_pool.tile([P, HID], F32, name=f"mi{t}", tag=f"mi{t}")
                         for t in range(2)]
                rms_norm_tiles(h_sb, nw2_sb, mi_sb, "rms2")
                miT = subC_pool.tile([P, (HID // P) * TOK], F32, name="miT", tag="miT")
                for t in range(2):
                    for k in range(HID // P):
                        transpose_128(miT[:, k * TOK + t * P:k * TOK + (t + 1) * P],
                                      mi_sb[t][:, k * P:(k + 1) * P])
                for t in range(2):
                    mib = sb2.tile([P, HID], BF16, name="mib", tag="mib")
                    nc.vector.tensor_copy(mib[:], mi_sb[t][:])
                    nc.sync.dma_start(agm_in[t * P:(t + 1) * P, :], mib[:])
                nc.gpsimd.collective_compute(
                    "AllGather", mybir.AluOpType.bypass,
                    ins=[agm_in[:]], outs=[agm_out[0:T, :]], replica_groups=RG)

                for t in range(2):
                    plog = ps.tile([P, E], F32, name="plog", tag="ps")
                    for k in range(HID // P):
                        nc.tensor.matmul(
                            out=plog[:],
                            lhsT=miT[:, k * TOK + t * P:k * TOK + (t + 1) * P],
                            rhs=wrout_sb[k][:],
                            start=(k == 0), stop=(k == HID // P - 1))
                    lmax = sb2.tile([P, 1], F32, name="lmax", tag="lmax")
                    nc.vector.reduce_max(lmax[:], plog[:], axis=mybir.AxisListType.X)
                    nlmax = sb2.tile([P, 1], F32, name="nlmax", tag="nlmax")
                    nc.vector.tensor_scalar(nlmax[:], lmax[:], -1.0, None,
                                            op0=mybir.AluOpType.mult)
                    pe_ = sb2.tile([P, E], F32, name="pexp", tag="pexp")
                    sume = sb2.tile([P, 1], F32, name="sume", tag="sume")
                    nc.scalar.activation(pe_[:], plog[:],
                                         mybir.ActivationFunctionType.Exp,
                                         bias=nlmax[:], accum_out=sume[:])
                    rse = sb2.tile([P, 1], F32, name="rse", tag="rse")
                    nc.vector.reciprocal(rse[:], sume[:])
                    probs = sb2.tile([P, E], F32, name="probs", tag="probs")
                    nc.vector.tensor_mul(probs[:], pe_[:], rse[:].to_broadcast([P, E]))
                    m8 = sb2.tile([P, 8], F32, name="m8", tag="m8")
                    nc.vector.max(out=m8[:], in_=probs[:])
                    s12 = sb2.tile([P, 1], F32, name="s12", tag="s12")
                    nc.vector.tensor_add(s12[:], m8[:, 0:1], m8[:, 1:2])
                    rs12 = sb2.tile([P, 1], F32, name="rs12", tag="rs12")
                    nc.vector.reciprocal(rs12[:], s12[:])
                    w12 = sb2.tile([P, 2], F32, name="w12", tag="w12")
                    nc.vector.tensor_mul(w12[:], m8[:, 0:2], rs12[:].to_broadcast([P, 2]))
                    acc = comb_sb[t]
                    mka = sb2.tile([P, E], F32, name="mka", tag="mka")
                    nc.vector.tensor_tensor(mka[:], probs[:],
                                            m8[:, 0:1].to_broadcast([P, E]),
                                            op=mybir.AluOpType.is_equal)
                    nc.vector.tensor_mul(acc[:], mka[:], w12[:, 0:1].to_broadcast([P, E]))
                    nc.vector.tensor_tensor(mka[:], probs[:],
                                            m8[:, 1:2].to_broadcast([P, E]),
                                            op=mybir.AluOpType.is_equal)
                    nc.vector.tensor_mul(mka[:], mka[:], w12[:, 1:2].to_broadcast([P, E]))
                    nc.vector.tensor_add(acc[:], acc[:], mka[:])
                    nc.sync.dma_start(agc_in[t * P:(t + 1) * P, :], acc[:])
                nc.gpsimd.collective_compute(
                    "AllGather", mybir.AluOpType.bypass,
                    ins=[agc_in[:]], outs=[agc_out[:]], replica_groups=RG)
                subC.__exit__(None, None, None)

            # ======================= MoE phase =======================
            with tc.tile_pool(name="moep", bufs=1) as moep, \
                 tc.tile_pool(name="moe2", bufs=2) as moe2:
                sel_sb = [moep.tile([P, E], F32, name=f"sel{e}", tag=f"sel{e}")
                          for e in range(EPL)]
                for e in range(EPL):
                    nc.sync.dma_start(sel_sb[e][:], sel_in[e])

                iota_i = moep.tile([16, T // 16], I32, name="iota_i", tag="iota_i")
                nc.gpsimd.iota(iota_i[:], pattern=[[16, T // 16]], base=0,
                               channel_multiplier=1)
                iota_f = moep.tile([16, T // 16], F32, name="iota_f", tag="iota_f")
                nc.vector.tensor_copy(iota_f[:], iota_i[:])

                idx_tiles = [[None] * CT for _ in range(EPL)]
                w_tiles = [[None] * CT for _ in range(EPL)]
                for e in range(EPL):
                    col_sb = moep.tile([P, KC], F32, name=f"colsb{e}", tag=f"colsb{e}")
                    for t in range(KC):
                        ctile = moe2.tile([P, E], F32, name="ctile", tag="ctile")
                        nc.sync.dma_start(ctile[:], agc_out[t * P:(t + 1) * P, :])
                        prod = moe2.tile([P, E], F32, name="cprod", tag="cprod")
                        nc.vector.tensor_mul(prod[:], ctile[:], sel_sb[e][:])
                        nc.vector.reduce_sum(col_sb[:, t:t + 1], prod[:],
                                             axis=mybir.AxisListType.X)
                    nc.sync.dma_start(colbuf[:].rearrange("(t p) -> p t", p=P),
                                      col_sb[:])
                    cw = moep.tile([16, T // 16 + CF], F32, name=f"cw{e}", tag=f"cw{e}")
                    nc.sync.dma_start(cw[:, 0:T // 16],
                                      colbuf[:].rearrange("(f p) -> p f", p=16))
                    nc.vector.memset(cw[:, T // 16:], 0.0)
                    msk = moep.tile([16, T // 16], F32, name=f"msk{e}", tag=f"msk{e}")
                    nc.vector.tensor_scalar(msk[:], cw[:, 0:T // 16], 0.0, None,
                                            op0=mybir.AluOpType.is_gt)
                    iin = moep.tile([16, T // 16 + CF], F32, name=f"iin{e}", tag=f"iin{e}")
                    t1 = sb2.tile([16, T // 16], F32, name="irt1", tag="irt1")
                    nc.vector.tensor_scalar(t1[:], iota_f[:], 1.0, None,
                                            op0=mybir.AluOpType.add)
                    nc.vector.tensor_mul(t1[:], t1[:], msk[:])
                    nc.vector.tensor_scalar(iin[:, 0:T // 16], t1[:], -1.0, None,
                                            op0=mybir.AluOpType.add)
                    nc.vector.memset(iin[:, T // 16:], float(SENT))
                    nc.vector.tensor_scalar(msk[:], msk[:], -1.0, None,
                                            op0=mybir.AluOpType.add)
                    nc.vector.tensor_add(cw[:, 0:T // 16], cw[:, 0:T // 16], msk[:])
                    # output sized 2*CAP: total found = n_real + CAP sentinels
                    # can reach ~680; only the first CAP entries are consumed
                    idx_c = moep.tile([16, 2 * CF], F32, name=f"idxc{e}", tag=f"idxc{e}")
                    w_c = moep.tile([16, 2 * CF], F32, name=f"wc{e}", tag=f"wc{e}")
                    nf = sb2.tile([1, 1], mybir.dt.uint32, name="nf", tag="nf")
                    nc.gpsimd.sparse_gather(idx_c[:], iin[:], num_found=nf[:])
                    nf2 = sb2.tile([1, 1], mybir.dt.uint32, name="nf2", tag="nf2")
                    nc.gpsimd.sparse_gather(w_c[:], cw[:], num_found=nf2[:])
                    nc.sync.dma_start(scr_idx[e].rearrange("(f p) -> p f", p=16),
                                      idx_c[:, 0:CF])
                    nc.sync.dma_start(scr_w[e].rearrange("(f p) -> p f", p=16),
                                      w_c[:, 0:CF])
                    for ct in range(CT):
                        fidx = moep.tile([P, 1], F32, name=f"fidx{e}_{ct}",
                                         tag=f"fidx{e}_{ct}")
                        nc.sync.dma_start(fidx[:],
                                          scr_idx[e, ct * P:(ct + 1) * P, None])
                        ii = moep.tile([P, 1], I32, name=f"ii{e}_{ct}",
                                       tag=f"ii{e}_{ct}")
                        nc.vector.tensor_copy(ii[:], fidx[:])
                        idx_tiles[e][ct] = ii
                        fw = moep.tile([P, 1], F32, name=f"fw{e}_{ct}",
                                       tag=f"fw{e}_{ct}")
                        nc.sync.dma_start(fw[:], scr_w[e, ct * P:(ct + 1) * P, None])
                        w_tiles[e][ct] = fw

                # expert weights: one tag-set per expert (serial reuse of space)
                wgu_sb = [[moep.tile([P, 2 * FF], BF16, name=f"wgu{e}_{k}",
                                     tag=f"wgu{e}_{k}") for k in range(HID // P)]
                          for e in range(EPL)]
                wdn_sb = [[moep.tile([P, HID], BF16, name=f"wdn{e}_{k}",
                                     tag=f"wdn{e}_{k}") for k in range(FF // P)]
                          for e in range(EPL)]
                for e in range(EPL):
                    for k in range(HID // P):
                        nc.sync.dma_start(wgu_sb[e][k][:],
                                          wgu_in[e, k * P:(k + 1) * P, :])
                    for k in range(FF // P):
                        nc.sync.dma_start(wdn_sb[e][k][:],
                                          wdn_in[e, k * P:(k + 1) * P, :])

                for e in range(EPL):
                    xgT = moep.tile([P, (HID // P) * CAP], BF16, name="xgT", tag="xgT")
                    wrow = moep.tile([1, CAP], F32, name="wrow", tag="wrow")
                    for ct in range(CT):
                        xg = moe2.tile([P, HID], BF16, name="xg", tag="xg")
                        nc.gpsimd.indirect_dma_start(
                            out=xg[:], out_offset=None,
                            in_=agm_out[:, :],
                            in_offset=bass.IndirectOffsetOnAxis(
                                ap=idx_tiles[e][ct][:, :1], axis=0))
                        for k in range(HID // P):
                            transpose_128(
                                xgT[:, k * CAP + ct * P:k * CAP + (ct + 1) * P],
                                xg[:, k * P:(k + 1) * P])
                        pwr = ps.tile([P, P], F32, name="pwr", tag="ps")
                        nc.tensor.matmul(out=pwr[:1, :P], lhsT=w_tiles[e][ct][:, :1],
                                         rhs=ident[:, :P], start=True, stop=True,
                                         is_transpose=True)
                        nc.vector.tensor_copy(wrow[:, ct * P:(ct + 1) * P],
                                              pwr[:1, :P])
                    pwb = ps.tile([P, CAP], F32, name="pwb", tag="ps")
                    nc.tensor.matmul(out=pwb[:], lhsT=ones_row[:, :P], rhs=wrow[:],
                                     start=True, stop=True)
                    wb = moep.tile([P, CAP], F32, name="wb", tag="wb")
                    nc.vector.tensor_copy(wb[:], pwb[:])

                    hT = moep.tile([P, (FF // P) * CAP], BF16, name="hT", tag="hT")
                    gsT = moep.tile([P, (FF // P) * CAP], BF16, name="gsT", tag="gsT")
                    for n in range(2 * FF // P):
                        pgu = ps_acc.tile([P, CAP], F32, name="pgu", tag="acc")
                        for k in range(HID // P):
                            nc.tensor.matmul(
                                out=pgu[:],
                                lhsT=wgu_sb[e][k][:, n * P:(n + 1) * P],
                                rhs=xgT[:, k * CAP:(k + 1) * CAP],
                                start=(k == 0), stop=(k == HID // P - 1))
                        if n < FF // P:
                            nc.scalar.activation(gsT[:, n * CAP:(n + 1) * CAP], pgu[:],
                                                 mybir.ActivationFunctionType.Silu)
                        else:
                            m = n - FF // P
                            tmp = sb2.tile([P, CAP], F32, name="hum", tag="hum")
                            nc.vector.tensor_mul(tmp[:], pgu[:],
                                                 gsT[:, m * CAP:(m + 1) * CAP])
                            nc.vector.tensor_mul(hT[:, m * CAP:(m + 1) * CAP],
                                                 tmp[:], wb[:])

                    for mo in range(HID // P):
                        pdn = ps_acc.tile([P, CAP], F32, name="pdn", tag="acc")
                        for k in range(FF // P):
                            nc.tensor.matmul(
                                out=pdn[:],
                                lhsT=wdn_sb[e][k][:, mo * P:(mo + 1) * P],
                                rhs=hT[:, k * CAP:(k + 1) * CAP],
                                start=(k == 0), stop=(k == FF // P - 1))
                        # reuse gsT cols as bf16 scratch for the feature-major result
                        nc.vector.tensor_copy(gsT[:, mo * CAP:(mo + 1) * CAP], pdn[:])
                    for ct in range(CT):
                        og = moe2.tile([P, HID], BF16, name="og", tag="og")
                        for k in range(HID // P):
                            transpose_128(og[:, k * P:(k + 1) * P],
                                          gsT[:, k * CAP + ct * P:k * CAP + (ct + 1) * P])
                        if e == 1:
                            prev = moe2.tile([P, HID], BF16, name="prev", tag="prev")
                            nc.gpsimd.indirect_dma_start(
                                out=prev[:], out_offset=None,
                                in_=partial[:, :],
                                in_offset=bass.IndirectOffsetOnAxis(
                                    ap=idx_tiles[e][ct][:, :1], axis=0))
                            nc.vector.tensor_add(og[:], og[:], prev[:])
                        nc.gpsimd.indirect_dma_start(
                            out=partial[:, :],
                            out_offset=bass.IndirectOffsetOnAxis(
                                ap=idx_tiles[e][ct][:, :1], axis=0),
                            in_=og[:], in_offset=None)

                # combine across cores; rank r receives its own 256-token chunk
                nc.gpsimd.collective_compute(
                    "ReduceScatter", mybir.AluOpType.add,
                    ins=[partial[0:T, :]], outs=[rs_out[:]], replica_groups=RG)
                for t in range(2):
                    rso = moe2.tile([P, HID], BF16, name="rso", tag="rso")
                    nc.sync.dma_start(rso[:], rs_out[t * P:(t + 1) * P, :])
                    oo = moe2.tile([P, HID], F32, name="oo", tag="oo")
                    nc.vector.tensor_add(oo[:], h_sb[t][:], rso[:])
                    nc.sync.dma_start(out_ext[t * P:(t + 1) * P, :], oo[:])

    # raw Bass skips Bacc's library-load + extended-inst codegen passes;
    # sparse_gather needs both (gpsimd ucode library + .instr bytes)
    from concourse import bacc as _bacc
    _bacc.Bacc.insert_library_loads(nc)
    _bacc.Bacc.codegen_inst_isa_subclasses(nc)
    return nc


_ROPE_CACHE = None


def _host_consts():
    global _ROPE_CACHE
    if _ROPE_CACHE is None:
        inv = 1.0 / (10000.0 ** (np.arange(0, HD, 2, dtype=np.float64) / HD))
        f = np.arange(T, dtype=np.float64)[:, None] * inv[None, :]
        _ROPE_CACHE = (np.cos(f).astype(np.float32), np.sin(f).astype(np.float32))
    return _ROPE_CACHE


def _to_bf16(a):
    import ml_dtypes
    return np.ascontiguousarray(a.astype(ml_dtypes.bfloat16))


def _make_in_maps(x, norm1_w, w_qkv, w_out, norm2_w, w_router, w_gate_up, w_down):
    cos_t, sin_t = _host_consts()
    x2 = np.ascontiguousarray(np.asarray(x, dtype=np.float32).reshape(T, HID))
    wq = np.ascontiguousarray(np.asarray(w_qkv, np.float32))
    wo = np.ascontiguousarray(np.asarray(w_out, np.float32))
    wr = np.ascontiguousarray(np.asarray(w_router, np.float32))
    nw1 = np.ascontiguousarray(np.broadcast_to(np.asarray(norm1_w, np.float32), (P, HID)))
    nw2 = np.ascontiguousarray(np.broadcast_to(np.asarray(norm2_w, np.float32), (P, HID)))
    kpos = np.arange(T)
    in_maps = []
    for r in range(NCORES):
        lo = r * TOK
        qpos = np.arange(lo, lo + TOK)
        maskT = (kpos.reshape(KC, P, 1) <= qpos.reshape(1, 1, TOK))
        sel = np.zeros((EPL, P, E), dtype=np.float32)
        for e in range(EPL):
            sel[e, :, EPL * r + e] = 1.0
        in_maps.append({
            "x_chunk": x2[lo:lo + TOK],
            "w_qkv": wq,
            "w_out": wo,
            "w_router": wr,
            "w_gu": _to_bf16(np.asarray(w_gate_up[EPL * r:EPL * (r + 1)], np.float32)),
            "w_dn": _to_bf16(np.asarray(w_down[EPL * r:EPL * (r + 1)], np.float32)),
            "nw1": nw1,
            "nw2": nw2,
            "rope_cos": np.ascontiguousarray(cos_t[lo:lo + TOK]),
            "rope_sin": np.ascontiguousarray(sin_t[lo:lo + TOK]),
            "maskT": _to_bf16(maskT.astype(np.float32)),
            "sel": sel,
        })
    return in_maps


def kernel(x, norm1_w, w_qkv, w_out, norm2_w, w_router, w_gate_up, w_down, **run_kwargs):
    B, S, _ = x.shape
    assert (B, S) == (1, T)
    nc = _build_program()
    in_maps = _make_in_maps(x, norm1_w, w_qkv, w_out, norm2_w, w_router,
                            w_gate_up, w_down)
    res = run_bass_kernel_spmd(nc, in_maps, list(range(NCORES)), **run_kwargs)
    chunks = [np.asarray(res.results[r]["out_chunk"]) for r in range(NCORES)]
    out = np.concatenate(chunks, axis=0).reshape(1, T, HID).astype(np.float32)
    if run_kwargs:
        return out, res
    return out


if __name__ == "__main__":
    _build_program()
    print("program built OK")

